# revision 1
# baseline (speedup 1.0000x reference)
"""K-means step kernel for Trainium2 (8 NeuronCores, data-parallel over n).

scores[n,k] = ||c_k||^2 - 2 x_n.c_k ; assign = argmin_k ; new centroids =
segment-mean.  Strategy per core (n_loc = n/8 rows):
  mm1: dot' = x @ (-2C)^T via 3-term fp16 split (x_hi*C_hi + x_hi*C_lo +
       x_lo*C_hi) accumulated in fp32 PSUM -> fp32-accurate scores at
       1 cyc/row instead of fp32 matmul's 4.
  DVE: tensor_tensor_reduce fuses (dot' + c_sq) with a running min ->
       scores in SBUF + per-row min; tensor_scalar is_equal -> one-hot (f16).
  mm2: partial_sums = onehot^T @ [x_hi, 1] + onehot^T @ [x_lo, 0]
       (exact fp32 sums + exact counts in the appended column).
  Host: sum the 8 per-core partials, divide, keep old centroid where empty.
"""

import numpy as np

import concourse.bass as bass
import concourse.mybir as mybir
import concourse.tile as tile
from concourse.bass_utils import run_bass_kernel_spmd
from concourse.vector_clock import ScopedClock

# ---------------------------------------------------------------------------
# Workaround: walrus rejects >1 sem wait on CTRL (drain/nop) instructions.
# Split the TileContext exit-drain's waits across one NOP per wait.
_MAXW = 1


def _patched_drain_and_barrier(self, tick_clock, wait_clock):
    nc = self.nc
    drain_inst = nc.sync.drain()
    wait_clock.add_sem_waits(
        drain_inst.ins, ScopedClock({None: tick_clock.global_clock})
    )
    si = drain_inst.ins.sync_info
    waits = list(si.on_wait) if si and si.on_wait else []
    if len(waits) > _MAXW:
        drain_inst.ins.sync_info = mybir.SyncInfo(
            on_wait=waits[:_MAXW], on_update=list(si.on_update or [])
        )
        rest = waits[_MAXW:]
        for i in range(0, len(rest), _MAXW):
            nop = nc.sync.nop()
            nop.ins.sync_info = mybir.SyncInfo(
                on_wait=rest[i : i + _MAXW], on_update=[]
            )
    nc.all_engine_barrier()
    popped = nc._tile_sem_poison_stack.pop()
    assert popped is self._sem_poison
    nc.clear_and_free_semaphores(list(self.sems.allocated().values()))
    nc.all_engine_barrier()


tile.TileContext._drain_and_barrier = _patched_drain_and_barrier

# This walrus build accepts only ONE sync wait per instruction, but Tile's
# scheduler emits several on phase joins.  Rewrite the BIR before compiling:
# excess waits move onto same-engine NOPs inserted just before the
# instruction (identical semantics: all waits still complete before it).
import json as _json

import concourse.bass2jax as _bass2jax

_orig_compile_bir = _bass2jax.compile_bir_kernel


def _split_waits_compile(bir_json, tmpdir, neff_name="file.neff"):
    j = _json.loads(bir_json)
    cnt = 0
    for f in j["functions"]:
        for bb in f["blocks"]:
            out = []
            for ins in bb["instructions"]:
                si = ins.get("sync_info")
                ow = (si or {}).get("on_wait") or []
                if len(ow) > 1:
                    for w in ow[:-1]:
                        cnt += 1
                        out.append(
                            {
                                "debug": ins.get("debug"),
                                "engine": ins["engine"],
                                "ins": [],
                                "outs": [],
                                "name": f"I-wsplit-{cnt}",
                                "opcode": "NoOp",
                                "sync_info": {"on_update": [], "on_wait": [w]},
                            }
                        )
                    si["on_wait"] = [ow[-1]]
                out.append(ins)
            bb["instructions"] = out
    return _orig_compile_bir(_json.dumps(j).encode(), tmpdir, neff_name=neff_name)


_bass2jax.compile_bir_kernel = _split_waits_compile
# ---------------------------------------------------------------------------

N_CORES = 8
P = 128
F16 = mybir.dt.float16
F32 = mybir.dt.float32
ADD = mybir.AluOpType.add
MIN = mybir.AluOpType.min
EQ = mybir.AluOpType.is_equal

_KERNEL_CACHE = {}


def build_kernel(n_loc, k, d, group=8, ps1_bufs=3, ps2_bufs=2, xt_bufs=3, sc_bufs=3, exact_mm2=True):
    ntiles = n_loc // P
    ndh = d // P            # 128-row halves of the contraction dim
    nq = k // 512           # 512-wide k quarters for mm1 (1 PSUM bank each)
    nchunks = k // P        # 128-row output chunks for mm2
    daug = d + 1

    nc = bass.Bass()
    xT_hi = nc.declare_dram_parameter("xT_hi", [d, n_loc], F16, isOutput=False)
    xT_lo = nc.declare_dram_parameter("xT_lo", [d, n_loc], F16, isOutput=False)
    xa_hi = nc.declare_dram_parameter("xa_hi", [n_loc, daug], F16, isOutput=False)
    xa_lo = nc.declare_dram_parameter("xa_lo", [n_loc, daug], F16, isOutput=False)
    ChiT = nc.declare_dram_parameter("ChiT", [d, k], F16, isOutput=False)
    CloT = nc.declare_dram_parameter("CloT", [d, k], F16, isOutput=False)
    csqb = nc.declare_dram_parameter("csqb", [P, k], F32, isOutput=False)
    out = nc.declare_dram_parameter("out", [k, daug], F32, isOutput=True)

    with tile.TileContext(nc) as tc:
        with (
            tc.tile_pool(name="consts", bufs=1) as consts,
            tc.tile_pool(name="xt", bufs=xt_bufs) as xtp,
            tc.tile_pool(name="xaug", bufs=2 * group + 2) as xap,
            tc.tile_pool(name="oh", bufs=2 * group + 2) as ohp,
            tc.tile_pool(name="sc", bufs=sc_bufs) as scp,
            tc.tile_pool(name="mp", bufs=6) as mp,
            tc.tile_pool(name="ps1", bufs=ps1_bufs, space="PSUM") as ps1,
            tc.tile_pool(name="ps2", bufs=ps2_bufs, space="PSUM") as ps2,
        ):
            chi = [consts.tile([P, k], F16, tag=f"chi{i}", name=f"chi{i}") for i in range(ndh)]
            clo = [consts.tile([P, k], F16, tag=f"clo{i}", name=f"clo{i}") for i in range(ndh)]
            for i in range(ndh):
                nc.sync.dma_start(out=chi[i], in_=ChiT[i * P : (i + 1) * P, :])
                nc.sync.dma_start(out=clo[i], in_=CloT[i * P : (i + 1) * P, :])
            csq = consts.tile([P, k], F32, tag="csq", name="csq")
            nc.sync.dma_start(out=csq, in_=csqb[:, :])
            acc = consts.tile([P, nchunks * daug], F32, tag="acc", name="acc")
            nc.vector.memset(acc, 0.0)

            def emit_mm2(pend):
                ohs, xhis, xlos = pend
                ng = len(ohs)
                for c in range(nchunks):
                    pc = ps2.tile([P, daug], F32, tag="ps2", name="pc")
                    for g in range(ng):
                        nc.tensor.matmul(
                            pc, ohs[g][:, c * P : (c + 1) * P], xhis[g],
                            start=(g == 0),
                            stop=(not exact_mm2 and g == ng - 1),
                        )
                    if exact_mm2:
                        for g in range(ng):
                            nc.tensor.matmul(
                                pc, ohs[g][:, c * P : (c + 1) * P], xlos[g],
                                start=False, stop=(g == ng - 1),
                            )
                    nc.vector.tensor_tensor(
                        acc[:, c * daug : (c + 1) * daug], pc,
                        acc[:, c * daug : (c + 1) * daug], op=ADD,
                    )

            pending = None
            cur = ([], [], [])
            for i in range(ntiles):
                xth = [xtp.tile([P, P], F16, tag=f"xth{j}", name=f"xth{j}") for j in range(ndh)]
                xtl = [xtp.tile([P, P], F16, tag=f"xtl{j}", name=f"xtl{j}") for j in range(ndh)]
                for j in range(ndh):
                    nc.sync.dma_start(
                        out=xth[j], in_=xT_hi[j * P : (j + 1) * P, i * P : (i + 1) * P]
                    )
                    nc.sync.dma_start(
                        out=xtl[j], in_=xT_lo[j * P : (j + 1) * P, i * P : (i + 1) * P]
                    )
                xh = xap.tile([P, daug], F16, tag="xah", name="xah")
                xl = xap.tile([P, daug], F16, tag="xal", name="xal")
                nc.sync.dma_start(out=xh, in_=xa_hi[i * P : (i + 1) * P, :])
                nc.sync.dma_start(out=xl, in_=xa_lo[i * P : (i + 1) * P, :])

                scores = scp.tile([P, k], F32, tag="scores", name="scores")
                m_prev = None
                for h in range(nq // 2):  # 1024-wide halves (2 banks PSUM)
                    ph = ps1.tile([P, 1024], F32, tag="ps1", name="ph")
                    for q in range(2):  # 512-wide accumulation groups
                        col = h * 1024 + q * 512
                        terms = []
                        for j in range(ndh):
                            terms.append((xth[j], chi[j]))
                            terms.append((xth[j], clo[j]))
                        for j in range(ndh):
                            terms.append((xtl[j], chi[j]))
                        for t, (w, cm) in enumerate(terms):
                            nc.tensor.matmul(
                                ph[:, q * 512 : (q + 1) * 512],
                                w, cm[:, col : col + 512],
                                start=(t == 0), stop=(t == len(terms) - 1),
                            )
                    mh = mp.tile([P, 1], F32, tag=f"m{h % 2}", name=f"mh{h % 2}")
                    nc.vector.tensor_tensor(
                        scores[:, h * 1024 : (h + 1) * 1024],
                        ph, csq[:, h * 1024 : (h + 1) * 1024], op=ADD,
                    )
                    nc.vector.tensor_reduce(
                        mh, scores[:, h * 1024 : (h + 1) * 1024],
                        axis=mybir.AxisListType.X, op=MIN,
                    )
                    if m_prev is not None:
                        m2 = mp.tile([P, 1], F32, tag="mfin", name="m2")
                        nc.vector.tensor_tensor(m2, mh, m_prev, op=MIN)
                        mh = m2
                    m_prev = mh
                oh_t = ohp.tile([P, k], F16, tag="oh", name="oh_t")
                nc.vector.tensor_scalar(
                    out=oh_t, in0=scores, scalar1=m_prev, scalar2=None, op0=EQ
                )
                cur[0].append(oh_t)
                cur[1].append(xh)
                cur[2].append(xl)

                if len(cur[0]) == group:
                    if pending is not None:
                        emit_mm2(pending)
                    pending = cur
                    cur = ([], [], [])
            if pending is not None:
                emit_mm2(pending)
            if cur[0]:
                emit_mm2(cur)

            for c in range(nchunks):
                nc.sync.dma_start(
                    out=out[c * P : (c + 1) * P, :],
                    in_=acc[:, c * daug : (c + 1) * daug],
                )
    return nc


def _prep_inputs(x, C):
    n, d = x.shape
    k = C.shape[0]
    n_loc = n // N_CORES

    Cp = -2.0 * C.astype(np.float64)
    c_sq = np.sum(C.astype(np.float64) ** 2, axis=1).astype(np.float32)
    Chi = Cp.astype(np.float16)
    Clo = (Cp - Chi.astype(np.float64)).astype(np.float16)
    ChiT = np.ascontiguousarray(Chi.T)
    CloT = np.ascontiguousarray(Clo.T)
    csqb = np.ascontiguousarray(np.broadcast_to(c_sq, (P, k)))

    xh = x.astype(np.float16)
    xl = (x.astype(np.float64) - xh.astype(np.float64)).astype(np.float16)
    ones = np.ones((n_loc, 1), np.float16)
    zeros = np.zeros((n_loc, 1), np.float16)

    in_maps = []
    for c in range(N_CORES):
        sl = slice(c * n_loc, (c + 1) * n_loc)
        in_maps.append(
            {
                "xT_hi": np.ascontiguousarray(xh[sl].T),
                "xT_lo": np.ascontiguousarray(xl[sl].T),
                "xa_hi": np.ascontiguousarray(np.concatenate([xh[sl], ones], 1)),
                "xa_lo": np.ascontiguousarray(np.concatenate([xl[sl], zeros], 1)),
                "ChiT": ChiT,
                "CloT": CloT,
                "csqb": csqb,
            }
        )
    return in_maps


def kernel(x, centroids, _trace=False):
    x = np.asarray(x, dtype=np.float32)
    C = np.asarray(centroids, dtype=np.float32)
    n, d = x.shape
    k = C.shape[0]
    n_loc = n // N_CORES

    key = (n_loc, k, d)
    if key not in _KERNEL_CACHE:
        _KERNEL_CACHE[key] = build_kernel(n_loc, k, d)
    nc = _KERNEL_CACHE[key]

    in_maps = _prep_inputs(x, C)
    res = run_bass_kernel_spmd(
        nc, in_maps, core_ids=list(range(N_CORES)), trace=_trace
    )

    total = np.zeros((k, d + 1), np.float64)
    for c in range(N_CORES):
        total += res.results[c]["out"].astype(np.float64)
    sums = total[:, :d]
    counts = total[:, d]
    means = (sums / np.maximum(counts, 1.0)[:, None]).astype(np.float32)
    out = np.where(counts[:, None] > 0, means, C)
    if _trace:
        kernel._last_result = res
    return out.astype(np.float32)



# revision 7
# speedup vs baseline: 1.8141x; 1.8141x over previous
"""K-means step kernel for Trainium2 (8 NeuronCores, data-parallel over n).

scores[n,k] = ||c_k||^2 - 2 x_n.c_k ; assign = argmin_k ; new centroids =
segment-mean.  Per core (n_loc = n/8 rows, 128 row-tiles):

  mm1:  psum = xh16 @ Ch16 (fp16, 1 cyc/row)
        + DoubleRow fp8 corrections  xl8(e5m2) @ C8(e4m3)  and
          xh8(e4m3) @ Cl8(e5m2)   (0.5 cyc/row, d-chunk pairs as k-tiles)
        -> product error ~2^-14, argmin matches fp32 reference on all but
           ~10 near-tie rows (rel err ~5e-3 on the final output).
  csq:  p-flavor tiles fold csq into PSUM via a K=2 fp16 matmul
        (ones[2,rows] x [csq_hi; csq_lo]); d-flavor tiles add csq on DVE.
        The flavor split load-balances PE vs DVE.
  min:  DVE tensor_reduce; Act computes s' = scores - min (exact near 0),
        then onehot = Relu(1 - 1e6*s') -> exact 0/1 in fp8e4.
  mm2:  partials = onehot^T @ [x8hi|1] + onehot^T @ [x8lo|0] with fp8
        DoubleRow (row-chunk pairs as k-tiles, onehot stationary M=128).
  Host: sum the 8 per-core partials, divide, keep old centroid for empties.
"""

import numpy as np
import ml_dtypes

import concourse.bass as bass
import concourse.mybir as mybir
import concourse.tile as tile
from concourse.bass_utils import run_bass_kernel_spmd
from concourse.vector_clock import ScopedClock

# ---------------------------------------------------------------------------
# Workaround: walrus rejects >1 sem wait on CTRL (drain/nop) instructions.
# Split the TileContext exit-drain's waits across one NOP per wait.
_MAXW = 1


def _patched_drain_and_barrier(self, tick_clock, wait_clock):
    nc = self.nc
    drain_inst = nc.sync.drain()
    wait_clock.add_sem_waits(
        drain_inst.ins, ScopedClock({None: tick_clock.global_clock})
    )
    si = drain_inst.ins.sync_info
    waits = list(si.on_wait) if si and si.on_wait else []
    if len(waits) > _MAXW:
        drain_inst.ins.sync_info = mybir.SyncInfo(
            on_wait=waits[:_MAXW], on_update=list(si.on_update or [])
        )
        rest = waits[_MAXW:]
        for i in range(0, len(rest), _MAXW):
            nop = nc.sync.nop()
            nop.ins.sync_info = mybir.SyncInfo(
                on_wait=rest[i : i + _MAXW], on_update=[]
            )
    nc.all_engine_barrier()
    popped = nc._tile_sem_poison_stack.pop()
    assert popped is self._sem_poison
    nc.clear_and_free_semaphores(list(self.sems.allocated().values()))
    nc.all_engine_barrier()


tile.TileContext._drain_and_barrier = _patched_drain_and_barrier

# This walrus build accepts only ONE sync wait per instruction, but Tile's
# scheduler emits several on phase joins.  Rewrite the BIR before compiling:
# excess waits move onto same-engine NOPs inserted just before the
# instruction (identical semantics: all waits still complete before it).
import json as _json

import concourse.bass2jax as _bass2jax

_orig_compile_bir = _bass2jax.compile_bir_kernel


def _split_waits_compile(bir_json, tmpdir, neff_name="file.neff"):
    j = _json.loads(bir_json)
    cnt = 0
    for f in j["functions"]:
        for bb in f["blocks"]:
            out = []
            for ins in bb["instructions"]:
                si = ins.get("sync_info")
                ow = (si or {}).get("on_wait") or []
                if len(ow) > 1:
                    for w in ow[:-1]:
                        cnt += 1
                        out.append(
                            {
                                "debug": ins.get("debug"),
                                "engine": ins["engine"],
                                "ins": [],
                                "outs": [],
                                "name": f"I-wsplit-{cnt}",
                                "opcode": "NoOp",
                                "sync_info": {"on_update": [], "on_wait": [w]},
                            }
                        )
                    si["on_wait"] = [ow[-1]]
                out.append(ins)
            bb["instructions"] = out
    return _orig_compile_bir(_json.dumps(j).encode(), tmpdir, neff_name=neff_name)


_bass2jax.compile_bir_kernel = _split_waits_compile
# ---------------------------------------------------------------------------

N_CORES = 8
P = 128
F8E4 = mybir.dt.float8e4
F8E5 = mybir.dt.float8e5
F16 = mybir.dt.float16
F32 = mybir.dt.float32
ADD = mybir.AluOpType.add
MIN = mybir.AluOpType.min
MULT = mybir.AluOpType.mult
DR = mybir.MatmulPerfMode.DoubleRow
AF = mybir.ActivationFunctionType

f8e4 = ml_dtypes.float8_e4m3
f8e5 = ml_dtypes.float8_e5m2

BIG = 1.0e6  # relu onehot slope; min nonzero score gap is ~1.5e-5 (f32 ulp)

_KERNEL_CACHE = {}


def build_kernel(n_loc, k, d, p_num=9, p_den=16, group=8):
    """p_num/p_den: fraction of row-tiles that fold csq on the PE (p-flavor)."""
    ntiles = n_loc // P          # 128-row tiles
    npairs = ntiles // 2         # mm2 row-chunk pairs
    nkc = k // P                 # 128-wide centroid chunks (mm2 output tiles)
    daug = d + 1

    nc = bass.Bass()
    # tile-major x layouts (>=512B per partition row for fast DMA)
    XH = nc.declare_dram_parameter("XH", [ntiles, P, 2, P], F16, isOutput=False)
    # [tile, dpart, which(0=xl8 e5m2, 1=xh8 e4m3-bitcast), dchunk, row]
    X8 = nc.declare_dram_parameter("X8", [ntiles, P, 2, 2, P], F8E5, isOutput=False)
    # [pair, row, t(chunk-in-pair), which(0=hi,1=lo), col]
    XA8 = nc.declare_dram_parameter("XA8", [npairs, P, 2, 2, daug], F8E4,
                                    isOutput=False)
    CH = nc.declare_dram_parameter("CH", [2, P, k], F16, isOutput=False)
    C8P = nc.declare_dram_parameter("C8P", [P, 2, k], F8E4, isOutput=False)
    CL8P = nc.declare_dram_parameter("CL8P", [P, 2, k], F8E5, isOutput=False)
    CSQB = nc.declare_dram_parameter("CSQB", [P, k], F32, isOutput=False)
    CSQP = nc.declare_dram_parameter("CSQP", [2, k], F16, isOutput=False)
    ONES = nc.declare_dram_parameter("ONES", [2, P], F16, isOutput=False)
    OUT = nc.declare_dram_parameter("out", [k, daug], F32, isOutput=True)

    with tile.TileContext(nc) as tc:
        with (
            tc.tile_pool(name="consts", bufs=1) as consts,
            tc.tile_pool(name="xh", bufs=3) as xhp,
            tc.tile_pool(name="x8", bufs=3) as x8p,
            tc.tile_pool(name="xa", bufs=2 * group + 2) as xap,
            tc.tile_pool(name="oh", bufs=2 * group + 2) as ohp,
            tc.tile_pool(name="sc", bufs=3) as scp,
            tc.tile_pool(name="sh", bufs=3) as shp,
            tc.tile_pool(name="mm", bufs=8) as mmp,
            tc.tile_pool(name="ps1", bufs=3, space="PSUM") as ps1,
            tc.tile_pool(name="ps2", bufs=2, space="PSUM") as ps2,
        ):
            ch = [consts.tile([P, k], F16, tag=f"ch{j}", name=f"ch{j}")
                  for j in range(2)]
            for j in range(2):
                nc.sync.dma_start(out=ch[j], in_=CH[j])
            c8p = consts.tile([P, 2, k], F8E4, tag="c8p", name="c8p")
            cl8p = consts.tile([P, 2, k], F8E5, tag="cl8p", name="cl8p")
            csqb = consts.tile([P, k], F32, tag="csqb", name="csqb")
            csqp = consts.tile([2, k], F16, tag="csqp", name="csqp")
            ones = consts.tile([2, P], F16, tag="ones", name="ones")
            for t_, d_ in [(c8p, C8P), (cl8p, CL8P), (csqb, CSQB),
                           (csqp, CSQP), (ones, ONES)]:
                nc.sync.dma_start(out=t_, in_=d_[:])
            acc = consts.tile([P, nkc * daug], F32, tag="acc", name="acc")
            nc.vector.memset(acc, 0.0)

            def emit_mm2(pend):
                ohs, xas = pend
                ng = len(ohs)
                for c in range(nkc):
                    pc = ps2.tile([P, daug], F32, tag="ps2", name="pc")
                    for g in range(ng):
                        oh_s = ohs[g][:, :, c * P : (c + 1) * P]
                        for hl in range(2):
                            for c0, c1 in ((0, P), (P, daug)):
                                # one start per psum tile: HW/CoreSim zero the
                                # whole 2KB zero-region, covering both c0 groups
                                nc.tensor.matmul(
                                    pc[:, c0:c1],
                                    oh_s,
                                    xas[g][:, :, hl, c0:c1],
                                    start=(g == 0 and hl == 0 and c0 == 0),
                                    stop=(g == ng - 1 and hl == 1),
                                    perf_mode=DR,
                                    skip_group_check=True,
                                )
                    nc.vector.tensor_tensor(
                        acc[:, c * daug : (c + 1) * daug], pc,
                        acc[:, c * daug : (c + 1) * daug], op=ADD,
                    )

            pending = None
            cur = ([], [])
            oh_pair = None
            for i in range(ntiles):
                p_flavor = (i * p_num) // p_den != ((i + 1) * p_num) // p_den

                xh = xhp.tile([P, 2, P], F16, tag="xh", name="xh")
                nc.sync.dma_start(out=xh, in_=XH[i])
                x8 = x8p.tile([P, 2, 2, P], F8E5, tag="x8", name="x8")
                nc.sync.dma_start(out=x8, in_=X8[i])
                if i % 2 == 0:
                    xa = xap.tile([P, 2, 2, daug], F8E4, tag="xa", name="xa")
                    nc.sync.dma_start(out=xa, in_=XA8[i // 2])
                    oh_pair = ohp.tile([P, 2, k], F8E4, tag="oh", name="oh")

                phs = []
                for h in range(2):
                    ph = ps1.tile([P, 1024], F32, tag="ps1", name="ph")
                    base = h * 1024
                    for g in range(2):
                        col = base + g * 512
                        for j in range(2):
                            nc.tensor.matmul(
                                ph[:, g * 512 : (g + 1) * 512],
                                xh[:, j, :], ch[j][:, col : col + 512],
                                start=(j == 0), stop=False,
                                skip_group_check=True,
                            )
                    if p_flavor:
                        for g in range(2):
                            col = base + g * 512
                            nc.tensor.matmul(
                                ph[:, g * 512 : (g + 1) * 512],
                                ones, csqp[:, col : col + 512],
                                start=False, stop=False,
                                skip_group_check=True,
                            )
                    for q in range(4):
                        col = base + q * 256
                        qs = slice(q * 256, (q + 1) * 256)
                        nc.tensor.matmul(
                            ph[:, qs], x8[:, 0, :, :],
                            c8p[:, :, col : col + 256],
                            start=False, stop=False,
                            perf_mode=DR, skip_group_check=True,
                        )
                        nc.tensor.matmul(
                            ph[:, qs], x8[:, 1, :, :].bitcast(F8E4),
                            cl8p[:, :, col : col + 256],
                            start=False, stop=(q % 2 == 1),
                            perf_mode=DR, skip_group_check=True,
                        )
                    phs.append(ph)

                tneg = mmp.tile([P, 1], F32, tag="tneg", name="tneg")
                if p_flavor:
                    m0 = mmp.tile([P, 1], F32, tag="m0", name="m0")
                    m1 = mmp.tile([P, 1], F32, tag="m1", name="m1")
                    m2 = mmp.tile([P, 1], F32, tag="m2", name="m2")
                    nc.vector.tensor_reduce(m0, phs[0], axis=mybir.AxisListType.X,
                                            op=MIN)
                    nc.vector.tensor_reduce(m1, phs[1], axis=mybir.AxisListType.X,
                                            op=MIN)
                    nc.vector.tensor_tensor(m2, m0, m1, op=MIN)
                    nc.vector.tensor_scalar(out=tneg, in0=m2, scalar1=-1.0,
                                            scalar2=None, op0=MULT)
                    s16 = shp.tile([P, k], F16, tag="s16", name="s16")
                    for h in range(2):
                        nc.scalar.activation(s16[:, h * 1024 : (h + 1) * 1024],
                                             phs[h], AF.Identity, bias=tneg,
                                             scale=1.0)
                else:
                    scores = scp.tile([P, k], F32, tag="scores", name="scores")
                    for h in range(2):
                        nc.vector.tensor_tensor(
                            scores[:, h * 1024 : (h + 1) * 1024], phs[h],
                            csqb[:, h * 1024 : (h + 1) * 1024], op=ADD,
                        )
                    m2 = mmp.tile([P, 1], F32, tag="m2", name="m2")
                    nc.vector.tensor_reduce(m2, scores, axis=mybir.AxisListType.X,
                                            op=MIN)
                    nc.vector.tensor_scalar(out=tneg, in0=m2, scalar1=-1.0,
                                            scalar2=None, op0=MULT)
                    s16 = shp.tile([P, k], F16, tag="s16", name="s16")
                    nc.scalar.activation(s16, scores, AF.Identity, bias=tneg,
                                         scale=1.0)

                nc.scalar.activation(oh_pair[:, i % 2, :], s16, AF.Relu,
                                     bias=1.0, scale=-BIG)

                if i % 2 == 1:
                    cur[0].append(oh_pair)
                    cur[1].append(xa)
                    if len(cur[0]) == group:
                        if pending is not None:
                            emit_mm2(pending)
                        pending = cur
                        cur = ([], [])
            if pending is not None:
                emit_mm2(pending)
            if cur[0]:
                emit_mm2(cur)

            for c in range(nkc):
                nc.sync.dma_start(
                    out=OUT[c * P : (c + 1) * P, :],
                    in_=acc[:, c * daug : (c + 1) * daug],
                )
    return nc


def _prep_inputs(x, C):
    n, d = x.shape
    k = C.shape[0]
    n_loc = n // N_CORES
    ntiles = n_loc // P
    npairs = ntiles // 2
    daug = d + 1

    x64 = x.astype(np.float64)
    Cp = (-2.0 * C.astype(np.float64)).astype(np.float32)
    csq = np.sum(C.astype(np.float64) ** 2, axis=1).astype(np.float32)

    # fp16 / fp8 splits
    xh16 = x.astype(np.float16)
    xl = (x64 - xh16.astype(np.float64)).astype(np.float32)
    xl8 = xl.astype(f8e5)                               # (n, d) e5m2
    xh8 = x.astype(f8e4)                                # (n, d) e4m3
    Ch16 = Cp.astype(np.float16)
    Cl = (Cp.astype(np.float64) - Ch16.astype(np.float64)).astype(np.float32)
    C8 = Cp.astype(f8e4)
    Cl8 = Cl.astype(f8e5)
    csq_hi = csq.astype(np.float16)
    csq_lo = (csq.astype(np.float64) - csq_hi.astype(np.float64)).astype(
        np.float32).astype(np.float16)

    # mm2 operands
    xa_hi = x.astype(f8e4)
    xa_lo = (x64 - xa_hi.astype(np.float64)).astype(np.float32).astype(f8e4)

    # constant (per-core-identical) buffers
    ChT = np.ascontiguousarray(Ch16.T)                  # (d, k)
    CH = np.ascontiguousarray(ChT.reshape(2, P, k))     # [dchunk, dpart, k]
    C8T = np.ascontiguousarray(C8.T).reshape(2, P, k)   # [dchunk, dpart, k]
    C8P = np.ascontiguousarray(C8T.transpose(1, 0, 2))  # [dpart, dchunk, k]
    Cl8T = np.ascontiguousarray(Cl8.T).reshape(2, P, k)
    CL8P = np.ascontiguousarray(Cl8T.transpose(1, 0, 2))
    CSQB = np.ascontiguousarray(np.broadcast_to(csq, (P, k))).astype(np.float32)
    CSQP = np.ascontiguousarray(np.stack([csq_hi, csq_lo]))  # [2, k]
    ONES = np.ones((2, P), np.float16)

    in_maps = []
    for c in range(N_CORES):
        sl = slice(c * n_loc, (c + 1) * n_loc)
        # XH: [tile, dpart, dchunk, row]
        xh_c = xh16[sl].T.reshape(2, P, ntiles, P)          # [dchunk,dpart,t,r]
        XH = np.ascontiguousarray(xh_c.transpose(2, 1, 0, 3))
        # X8: [tile, dpart, which, dchunk, row] (e5m2 buffer, xh8 bitcast)
        xl8_c = xl8[sl].T.reshape(2, P, ntiles, P).transpose(2, 1, 0, 3)
        xh8_c = xh8[sl].T.reshape(2, P, ntiles, P).transpose(2, 1, 0, 3)
        X8 = np.empty((ntiles, P, 2, 2, P), dtype=f8e5)
        X8[:, :, 0] = xl8_c
        X8[:, :, 1] = xh8_c.view(np.uint8).view(f8e5)
        # XA8: [pair, row, t, which, col]
        XA8 = np.zeros((npairs, P, 2, 2, daug), dtype=f8e4)
        xa_hi_c = xa_hi[sl].reshape(npairs, 2, P, d).transpose(0, 2, 1, 3)
        xa_lo_c = xa_lo[sl].reshape(npairs, 2, P, d).transpose(0, 2, 1, 3)
        XA8[:, :, :, 0, :d] = xa_hi_c
        XA8[:, :, :, 1, :d] = xa_lo_c
        XA8[:, :, :, 0, d] = 1.0
        in_maps.append(
            {
                "XH": XH,
                "X8": X8,
                "XA8": XA8,
                "CH": CH,
                "C8P": C8P,
                "CL8P": CL8P,
                "CSQB": CSQB,
                "CSQP": CSQP,
                "ONES": ONES,
            }
        )
    return in_maps


def kernel(x, centroids, _trace=False):
    x = np.asarray(x, dtype=np.float32)
    C = np.asarray(centroids, dtype=np.float32)
    n, d = x.shape
    k = C.shape[0]
    n_loc = n // N_CORES

    key = (n_loc, k, d)
    if key not in _KERNEL_CACHE:
        _KERNEL_CACHE[key] = build_kernel(n_loc, k, d)
    nc = _KERNEL_CACHE[key]

    in_maps = _prep_inputs(x, C)
    res = run_bass_kernel_spmd(
        nc, in_maps, core_ids=list(range(N_CORES)), trace=_trace
    )

    total = np.zeros((k, d + 1), np.float64)
    for c in range(N_CORES):
        total += res.results[c]["out"].astype(np.float64)
    sums = total[:, :d]
    counts = total[:, d]
    means = (sums / np.maximum(counts, 1.0)[:, None]).astype(np.float32)
    out = np.where(counts[:, None] > 0, means, C)
    if _trace:
        kernel._last_result = res
    return out.astype(np.float32)


# revision 13
# speedup vs baseline: 1.8976x; 1.0460x over previous
"""K-means step kernel for Trainium2 (8 NeuronCores, data-parallel over n).

scores[n,k] = ||c_k||^2 - 2 x_n.c_k ; assign = argmin_k ; new centroids =
segment-mean.  Per core (n_loc = n/8 rows, 128 row-tiles):

  mm1:  psum = xh16 @ Ch16 (fp16, 1 cyc/row)
        + DoubleRow fp8 corrections  xl8(e5m2) @ C8(e4m3)  and
          xh8(e4m3) @ Cl8(e5m2)   (0.5 cyc/row, d-chunk pairs as k-tiles)
        -> product error ~2^-14, argmin matches fp32 reference on all but
           ~10 near-tie rows (rel err ~5e-3 on the final output).
  csq:  p-flavor tiles fold csq into PSUM via a K=2 fp16 matmul
        (ones[2,rows] x [csq_hi; csq_lo]); d-flavor tiles add csq on DVE.
        The flavor split load-balances PE vs DVE.
  min:  DVE tensor_reduce; Act computes s' = scores - min (exact near 0),
        then onehot = Relu(1 - 1e6*s') -> exact 0/1 in fp8e4.
  mm2:  partials = onehot^T @ [x8hi|1] + onehot^T @ [x8lo|0] with fp8
        DoubleRow (row-chunk pairs as k-tiles, onehot stationary M=128).
  Host: sum the 8 per-core partials, divide, keep old centroid for empties.
"""

import numpy as np
import ml_dtypes

import concourse.bass as bass
import concourse.mybir as mybir
import concourse.tile as tile
from concourse.bass_utils import run_bass_kernel_spmd
from concourse.vector_clock import ScopedClock

# ---------------------------------------------------------------------------
# Workaround: walrus rejects >1 sem wait on CTRL (drain/nop) instructions.
# Split the TileContext exit-drain's waits across one NOP per wait.
_MAXW = 1


def _patched_drain_and_barrier(self, tick_clock, wait_clock):
    nc = self.nc
    drain_inst = nc.sync.drain()
    wait_clock.add_sem_waits(
        drain_inst.ins, ScopedClock({None: tick_clock.global_clock})
    )
    si = drain_inst.ins.sync_info
    waits = list(si.on_wait) if si and si.on_wait else []
    if len(waits) > _MAXW:
        drain_inst.ins.sync_info = mybir.SyncInfo(
            on_wait=waits[:_MAXW], on_update=list(si.on_update or [])
        )
        rest = waits[_MAXW:]
        for i in range(0, len(rest), _MAXW):
            nop = nc.sync.nop()
            nop.ins.sync_info = mybir.SyncInfo(
                on_wait=rest[i : i + _MAXW], on_update=[]
            )
    nc.all_engine_barrier()
    popped = nc._tile_sem_poison_stack.pop()
    assert popped is self._sem_poison
    nc.clear_and_free_semaphores(list(self.sems.allocated().values()))
    nc.all_engine_barrier()


tile.TileContext._drain_and_barrier = _patched_drain_and_barrier

# This walrus build accepts only ONE sync wait per instruction, but Tile's
# scheduler emits several on phase joins.  Rewrite the BIR before compiling:
# excess waits move onto same-engine NOPs inserted just before the
# instruction (identical semantics: all waits still complete before it).
import json as _json

import concourse.bass2jax as _bass2jax

_orig_compile_bir = _bass2jax.compile_bir_kernel


def _split_waits_compile(bir_json, tmpdir, neff_name="file.neff"):
    j = _json.loads(bir_json)
    cnt = 0
    for f in j["functions"]:
        for bb in f["blocks"]:
            out = []
            for ins in bb["instructions"]:
                si = ins.get("sync_info")
                ow = (si or {}).get("on_wait") or []
                if len(ow) > 1:
                    for w in ow[:-1]:
                        cnt += 1
                        out.append(
                            {
                                "debug": ins.get("debug"),
                                "engine": ins["engine"],
                                "ins": [],
                                "outs": [],
                                "name": f"I-wsplit-{cnt}",
                                "opcode": "NoOp",
                                "sync_info": {"on_update": [], "on_wait": [w]},
                            }
                        )
                    si["on_wait"] = [ow[-1]]
                out.append(ins)
            bb["instructions"] = out
    return _orig_compile_bir(_json.dumps(j).encode(), tmpdir, neff_name=neff_name)


_bass2jax.compile_bir_kernel = _split_waits_compile
# ---------------------------------------------------------------------------

N_CORES = 8
P = 128
F8E4 = mybir.dt.float8e4
F8E5 = mybir.dt.float8e5
F16 = mybir.dt.float16
F32 = mybir.dt.float32
ADD = mybir.AluOpType.add
MIN = mybir.AluOpType.min
MULT = mybir.AluOpType.mult
DR = mybir.MatmulPerfMode.DoubleRow
AF = mybir.ActivationFunctionType

f8e4 = ml_dtypes.float8_e4m3
f8e5 = ml_dtypes.float8_e5m2

BIG = 1.0e6  # relu onehot slope; min nonzero score gap is ~1.5e-5 (f32 ulp)

_KERNEL_CACHE = {}


def build_kernel(n_loc, k, d, p_num=33, p_den=64, group=8):
    """p_num/p_den: fraction of row-tiles that fold csq on the PE (p-flavor)."""
    ntiles = n_loc // P          # 128-row tiles
    npairs = ntiles // 2         # mm2 row-chunk pairs
    nkc = k // P                 # 128-wide centroid chunks (mm2 output tiles)
    daug = d + 1

    nc = bass.Bass()
    # tile-major x layouts (>=512B per partition row for fast DMA)
    XH = nc.declare_dram_parameter("XH", [ntiles, P, 2, P], F16, isOutput=False)
    # [tile, dpart, which(0=xl8 e5m2, 1=xh8 e4m3-bitcast), dchunk, row]
    X8 = nc.declare_dram_parameter("X8", [ntiles, P, 2, 2, P], F8E5, isOutput=False)
    # [pair, row, t(chunk-in-pair), which(0=hi,1=lo), col]
    XA8 = nc.declare_dram_parameter("XA8", [npairs, P, 2, 2, daug], F8E4,
                                    isOutput=False)
    CH = nc.declare_dram_parameter("CH", [2, P, k], F16, isOutput=False)
    C8P = nc.declare_dram_parameter("C8P", [P, 2, k], F8E4, isOutput=False)
    CL8P = nc.declare_dram_parameter("CL8P", [P, 2, k], F8E5, isOutput=False)
    CSQB = nc.declare_dram_parameter("CSQB", [P, k], F32, isOutput=False)
    CSQP = nc.declare_dram_parameter("CSQP", [2, k], F16, isOutput=False)
    ONES = nc.declare_dram_parameter("ONES", [2, P], F16, isOutput=False)
    OUT = nc.declare_dram_parameter("out", [k, daug], F32, isOutput=True)

    with tile.TileContext(nc) as tc:
        with (
            tc.tile_pool(name="consts", bufs=1) as consts,
            tc.tile_pool(name="xh", bufs=4) as xhp,
            tc.tile_pool(name="x8", bufs=4) as x8p,
            tc.tile_pool(name="xa", bufs=2 * group + 2) as xap,
            tc.tile_pool(name="oh", bufs=2 * group + 2) as ohp,
            tc.tile_pool(name="sc", bufs=4) as scp,
            tc.tile_pool(name="sh", bufs=4) as shp,
            tc.tile_pool(name="mm", bufs=12) as mmp,
            tc.tile_pool(name="ps1", bufs=3, space="PSUM") as ps1,
            tc.tile_pool(name="ps2", bufs=2, space="PSUM") as ps2,
        ):
            ch = [consts.tile([P, k], F16, tag=f"ch{j}", name=f"ch{j}")
                  for j in range(2)]
            for j in range(2):
                nc.sync.dma_start(out=ch[j], in_=CH[j])
            c8p = consts.tile([P, 2, k], F8E4, tag="c8p", name="c8p")
            cl8p = consts.tile([P, 2, k], F8E5, tag="cl8p", name="cl8p")
            csqb = consts.tile([P, k], F32, tag="csqb", name="csqb")
            csqp = consts.tile([2, k], F16, tag="csqp", name="csqp")
            ones = consts.tile([2, P], F16, tag="ones", name="ones")
            for t_, d_ in [(c8p, C8P), (cl8p, CL8P), (csqb, CSQB),
                           (csqp, CSQP), (ones, ONES)]:
                nc.sync.dma_start(out=t_, in_=d_[:])
            acc = consts.tile([P, nkc * daug], F32, tag="acc", name="acc")
            nc.vector.memset(acc, 0.0)

            def emit_mm2_chunk(pend, c, sweep):
                ohs, xas = pend
                ng = len(ohs)
                pc = ps2.tile([P, daug], F32, tag="ps2", name="pc")
                for g in range(ng):
                    oh_s = ohs[g][:, :, c * P : (c + 1) * P]
                    for hl in range(2):
                        for c0, c1 in ((0, P), (P, daug)):
                            # one start per psum tile: HW/CoreSim zero the
                            # whole 2KB zero-region, covering both c0 groups
                            nc.tensor.matmul(
                                pc[:, c0:c1],
                                oh_s,
                                xas[g][:, :, hl, c0:c1],
                                start=(g == 0 and hl == 0 and c0 == 0),
                                stop=(g == ng - 1 and hl == 1),
                                perf_mode=DR,
                                skip_group_check=True,
                            )
                nc.vector.tensor_tensor(
                    acc[:, c * daug : (c + 1) * daug], pc,
                    acc[:, c * daug : (c + 1) * daug], op=ADD,
                )

            active = None        # group being emitted, 1 chunk per tile
            next_chunk = 0
            sweep = 0
            cur = ([], [])
            oh_pair = None
            for i in range(ntiles):
                p_flavor = (i * p_num) // p_den != ((i + 1) * p_num) // p_den

                xh = xhp.tile([P, 2, P], F16, tag="xh", name="xh")
                nc.sync.dma_start(out=xh, in_=XH[i])
                x8 = x8p.tile([P, 2, 2, P], F8E5, tag="x8", name="x8")
                nc.sync.dma_start(out=x8, in_=X8[i])
                if i % 2 == 0:
                    xa = xap.tile([P, 2, 2, daug], F8E4, tag="xa", name="xa")
                    nc.sync.dma_start(out=xa, in_=XA8[i // 2])
                    oh_pair = ohp.tile([P, 2, k], F8E4, tag="oh", name="oh")

                phs = []
                for h in range(2):
                    ph = ps1.tile([P, 1024], F32, tag="ps1", name="ph")
                    base = h * 1024
                    for g in range(2):
                        col = base + g * 512
                        for j in range(2):
                            nc.tensor.matmul(
                                ph[:, g * 512 : (g + 1) * 512],
                                xh[:, j, :], ch[j][:, col : col + 512],
                                start=(j == 0), stop=False,
                                skip_group_check=True,
                            )
                    if p_flavor:
                        for g in range(2):
                            col = base + g * 512
                            nc.tensor.matmul(
                                ph[:, g * 512 : (g + 1) * 512],
                                ones, csqp[:, col : col + 512],
                                start=False, stop=False,
                                skip_group_check=True,
                            )
                    for q in range(4):
                        col = base + q * 256
                        qs = slice(q * 256, (q + 1) * 256)
                        nc.tensor.matmul(
                            ph[:, qs], x8[:, 0, :, :],
                            c8p[:, :, col : col + 256],
                            start=False, stop=False,
                            perf_mode=DR, skip_group_check=True,
                        )
                        nc.tensor.matmul(
                            ph[:, qs], x8[:, 1, :, :].bitcast(F8E4),
                            cl8p[:, :, col : col + 256],
                            start=False, stop=(q % 2 == 1),
                            perf_mode=DR, skip_group_check=True,
                        )
                    phs.append(ph)

                tneg = mmp.tile([P, 1], F32, tag="tneg", name="tneg")
                if p_flavor:
                    m0 = mmp.tile([P, 1], F32, tag="m0", name="m0")
                    m1 = mmp.tile([P, 1], F32, tag="m1", name="m1")
                    nc.vector.tensor_reduce(m0, phs[0], axis=mybir.AxisListType.X,
                                            op=MIN, negate=True)
                    nc.vector.tensor_reduce(m1, phs[1], axis=mybir.AxisListType.X,
                                            op=MIN, negate=True)
                    nc.vector.tensor_tensor(tneg, m0, m1,
                                            op=mybir.AluOpType.max)
                    s16 = shp.tile([P, k], F16, tag="s16", name="s16")
                    for h in range(2):
                        nc.scalar.activation(s16[:, h * 1024 : (h + 1) * 1024],
                                             phs[h], AF.Identity, bias=tneg,
                                             scale=1.0)
                else:
                    scores = scp.tile([P, k], F32, tag="scores", name="scores")
                    for h in range(2):
                        nc.vector.tensor_tensor(
                            scores[:, h * 1024 : (h + 1) * 1024], phs[h],
                            csqb[:, h * 1024 : (h + 1) * 1024], op=ADD,
                        )
                    nc.vector.tensor_reduce(tneg, scores, axis=mybir.AxisListType.X,
                                             op=MIN, negate=True)
                    s16 = shp.tile([P, k], F16, tag="s16", name="s16")
                    nc.scalar.activation(s16, scores, AF.Identity, bias=tneg,
                                         scale=1.0)

                nc.scalar.activation(oh_pair[:, i % 2, :], s16, AF.Relu,
                                     bias=1.0, scale=-BIG)

                if i % 2 == 1:
                    cur[0].append(oh_pair)
                    cur[1].append(xa)
                    if len(cur[0]) == group:
                        assert active is None or next_chunk == nkc
                        active = cur
                        next_chunk = 0
                        cur = ([], [])
                # smooth mm2: one k-chunk of the previous group per tile
                if active is not None and next_chunk < nkc:
                    emit_mm2_chunk(active, next_chunk, sweep)
                    next_chunk += 1
                    if next_chunk == nkc:
                        sweep += 1
            while active is not None and next_chunk < nkc:
                emit_mm2_chunk(active, next_chunk, sweep)
                next_chunk += 1
                if next_chunk == nkc:
                    sweep += 1

            for c in range(nkc):
                nc.sync.dma_start(
                    out=OUT[c * P : (c + 1) * P, :],
                    in_=acc[:, c * daug : (c + 1) * daug],
                )
    return nc


def _prep_inputs(x, C):
    n, d = x.shape
    k = C.shape[0]
    n_loc = n // N_CORES
    ntiles = n_loc // P
    npairs = ntiles // 2
    daug = d + 1

    x64 = x.astype(np.float64)
    Cp = (-2.0 * C.astype(np.float64)).astype(np.float32)
    csq = np.sum(C.astype(np.float64) ** 2, axis=1).astype(np.float32)

    # fp16 / fp8 splits
    xh16 = x.astype(np.float16)
    xl = (x64 - xh16.astype(np.float64)).astype(np.float32)
    xl8 = xl.astype(f8e5)                               # (n, d) e5m2
    xh8 = x.astype(f8e4)                                # (n, d) e4m3
    Ch16 = Cp.astype(np.float16)
    Cl = (Cp.astype(np.float64) - Ch16.astype(np.float64)).astype(np.float32)
    C8 = Cp.astype(f8e4)
    Cl8 = Cl.astype(f8e5)
    csq_hi = csq.astype(np.float16)
    csq_lo = (csq.astype(np.float64) - csq_hi.astype(np.float64)).astype(
        np.float32).astype(np.float16)

    # mm2 operands
    xa_hi = x.astype(f8e4)
    xa_lo = (x64 - xa_hi.astype(np.float64)).astype(np.float32).astype(f8e4)

    # constant (per-core-identical) buffers
    ChT = np.ascontiguousarray(Ch16.T)                  # (d, k)
    CH = np.ascontiguousarray(ChT.reshape(2, P, k))     # [dchunk, dpart, k]
    C8T = np.ascontiguousarray(C8.T).reshape(2, P, k)   # [dchunk, dpart, k]
    C8P = np.ascontiguousarray(C8T.transpose(1, 0, 2))  # [dpart, dchunk, k]
    Cl8T = np.ascontiguousarray(Cl8.T).reshape(2, P, k)
    CL8P = np.ascontiguousarray(Cl8T.transpose(1, 0, 2))
    CSQB = np.ascontiguousarray(np.broadcast_to(csq, (P, k))).astype(np.float32)
    CSQP = np.ascontiguousarray(np.stack([csq_hi, csq_lo]))  # [2, k]
    ONES = np.ones((2, P), np.float16)

    in_maps = []
    for c in range(N_CORES):
        sl = slice(c * n_loc, (c + 1) * n_loc)
        # XH: [tile, dpart, dchunk, row]
        xh_c = xh16[sl].T.reshape(2, P, ntiles, P)          # [dchunk,dpart,t,r]
        XH = np.ascontiguousarray(xh_c.transpose(2, 1, 0, 3))
        # X8: [tile, dpart, which, dchunk, row] (e5m2 buffer, xh8 bitcast)
        xl8_c = xl8[sl].T.reshape(2, P, ntiles, P).transpose(2, 1, 0, 3)
        xh8_c = xh8[sl].T.reshape(2, P, ntiles, P).transpose(2, 1, 0, 3)
        X8 = np.empty((ntiles, P, 2, 2, P), dtype=f8e5)
        X8[:, :, 0] = xl8_c
        X8[:, :, 1] = xh8_c.view(np.uint8).view(f8e5)
        # XA8: [pair, row, t, which, col]
        XA8 = np.zeros((npairs, P, 2, 2, daug), dtype=f8e4)
        xa_hi_c = xa_hi[sl].reshape(npairs, 2, P, d).transpose(0, 2, 1, 3)
        xa_lo_c = xa_lo[sl].reshape(npairs, 2, P, d).transpose(0, 2, 1, 3)
        XA8[:, :, :, 0, :d] = xa_hi_c
        XA8[:, :, :, 1, :d] = xa_lo_c
        XA8[:, :, :, 0, d] = 1.0
        in_maps.append(
            {
                "XH": XH,
                "X8": X8,
                "XA8": XA8,
                "CH": CH,
                "C8P": C8P,
                "CL8P": CL8P,
                "CSQB": CSQB,
                "CSQP": CSQP,
                "ONES": ONES,
            }
        )
    return in_maps


def kernel(x, centroids, _trace=False):
    x = np.asarray(x, dtype=np.float32)
    C = np.asarray(centroids, dtype=np.float32)
    n, d = x.shape
    k = C.shape[0]
    n_loc = n // N_CORES

    key = (n_loc, k, d)
    if key not in _KERNEL_CACHE:
        _KERNEL_CACHE[key] = build_kernel(n_loc, k, d)
    nc = _KERNEL_CACHE[key]

    in_maps = _prep_inputs(x, C)
    res = run_bass_kernel_spmd(
        nc, in_maps, core_ids=list(range(N_CORES)), trace=_trace
    )

    total = np.zeros((k, d + 1), np.float64)
    for c in range(N_CORES):
        total += res.results[c]["out"].astype(np.float64)
    sums = total[:, :d]
    counts = total[:, d]
    means = (sums / np.maximum(counts, 1.0)[:, None]).astype(np.float32)
    out = np.where(counts[:, None] > 0, means, C)
    if _trace:
        kernel._last_result = res
    return out.astype(np.float32)
